# revision 17
# baseline (speedup 1.0000x reference)
"""Trainium2 Bass kernel for sparse autoencoder (topk masking).

  encoder:  pre = x @ W_enc.T + b_enc ; enc = relu(pre)
  topk:     per-row top-32 of enc kept, rest zeroed  -> encoded_sparse
  decoder:  dec = encoded_sparse @ W_dec.T + b_dec
  returns (encoded_sparse, dec)

Sharding: pure data-parallel over the batch dim across 8 NeuronCores
(1024 rows per core).  No collectives needed.

v6 pipeline per core (2 batch groups of 512 rows for cross-phase
overlap):
  Phase A(g): bf16 hi/lo-split matmuls (fp32-accurate), fused ReLU on
      ScalarE, park dense enc rows in DRAM.  While each relu chunk is
      still in SBUF, VectorE computes per-128-column-segment top-8
      candidates (no extra HBM traffic).
  Phase B(g): per row-tile: 7 small ops on the 1024-wide candidate
      array give the exact 32nd-largest value v32; then a streaming
      quarter-pipeline re-reads enc, applies the fused
      (enc >= v32) * enc mask in one VectorE pass, writes
      encoded_sparse, and PE-transposes to bf16 encT for the decoder.
  Phase C(g): dense bf16 decode, W_decT streamed, PSUM accumulation
      over 128 h-chunks, bias via K=1 matmul.
"""

import sys

sys.path.insert(0, "/opt/trn_rl_repo")

import numpy as np
import ml_dtypes

B, D, H, O, K = 8192, 1024, 16384, 1024, 32
NCORES = 8
BSH = B // NCORES  # 1024 rows per core
NG = 2             # batch groups per core
GB = BSH // NG     # 512 rows per group
NBT = GB // 128    # 4 row-tiles per group
HCH = 512          # encoder h-chunk
NHC = H // HCH     # 32
NDC = D // 128     # 8 contraction chunks
SEG = 128          # top-k candidate segment width
NSEG = H // SEG    # 128 segments -> 1024 candidates/row
HQ = H // 4        # phase-B quarter width (4096)

TRACE = False
LAST_RESULTS = {}

_cache = {}


def _build():
    import concourse.bass as bass  # noqa: F401
    import concourse.mybir as mybir
    import concourse.tile as tile
    from concourse import bacc
    from concourse.masks import make_identity
    from contextlib import ExitStack

    fp32 = mybir.dt.float32
    bf16 = mybir.dt.bfloat16
    RELU = mybir.ActivationFunctionType.Relu
    COPY = mybir.ActivationFunctionType.Copy

    nc = bacc.Bacc("TRN2", target_bir_lowering=False, debug=False,
                   num_devices=NCORES)

    xhi = nc.dram_tensor("xhi", [D, BSH], bf16, kind="ExternalInput").ap()
    xlo = nc.dram_tensor("xlo", [D, BSH], bf16, kind="ExternalInput").ap()
    whi = nc.dram_tensor("whi", [D, H], bf16, kind="ExternalInput").ap()
    wlo = nc.dram_tensor("wlo", [D, H], bf16, kind="ExternalInput").ap()
    bstack = nc.dram_tensor("bstack", [2, H], bf16, kind="ExternalInput").ap()
    wdecT = nc.dram_tensor("wdecT", [H, O], bf16, kind="ExternalInput").ap()
    bdec = nc.dram_tensor("bdec", [1, O], fp32, kind="ExternalInput").ap()
    enc_out = nc.dram_tensor("enc_sparse", [BSH, H], fp32,
                             kind="ExternalOutput").ap()
    dec_out = nc.dram_tensor("dec", [BSH, O], fp32,
                             kind="ExternalOutput").ap()

    with tile.TileContext(nc) as tc, ExitStack() as ctx:
        const = ctx.enter_context(tc.tile_pool(name="const", bufs=1))
        dram = ctx.enter_context(tc.tile_pool(name="dram", bufs=1,
                                              space="DRAM"))
        xpool = ctx.enter_context(tc.tile_pool(name="xTp", bufs=1))
        wpool = ctx.enter_context(tc.tile_pool(name="wenc", bufs=2))
        cpool = ctx.enter_context(tc.tile_pool(name="encch", bufs=3))
        candpool = ctx.enter_context(tc.tile_pool(name="cand", bufs=8))
        eqpool = ctx.enter_context(tc.tile_pool(name="encq", bufs=3))
        etpool = ctx.enter_context(tc.tile_pool(name="encT", bufs=2))
        mpool = ctx.enter_context(tc.tile_pool(name="m8", bufs=8))
        wdpool = ctx.enter_context(tc.tile_pool(name="wdec", bufs=3))
        ecpool = ctx.enter_context(tc.tile_pool(name="encTc", bufs=3))
        dpool = ctx.enter_context(tc.tile_pool(name="decout", bufs=2))
        psA = ctx.enter_context(tc.tile_pool(name="psA", bufs=2,
                                             space="PSUM"))
        psT = ctx.enter_context(tc.tile_pool(name="psT", bufs=2,
                                             space="PSUM"))
        psC = ctx.enter_context(tc.tile_pool(name="psC", bufs=4,
                                             space="PSUM"))

        ident = const.tile([128, 128], fp32)
        make_identity(nc, ident)
        ones2 = const.tile([2, 128], bf16)
        nc.vector.memset(ones2, 1.0)
        ones_f32 = const.tile([1, 128], fp32)
        nc.vector.memset(ones_f32, 1.0)
        bdec_sb = const.tile([1, O], fp32)
        nc.sync.dma_start(out=bdec_sb, in_=bdec)

        enc_dram = [dram.tile([GB, H], fp32, name=f"enc_dram{g}")
                    for g in range(NG)]
        encT_dram = [[dram.tile([H // 4, GB], bf16,
                                name=f"encT_dram{g}_{q}")
                      for q in range(4)] for g in range(NG)]

        for g in range(NG):
            gc = slice(g * GB, (g + 1) * GB)  # this group's batch columns
            # ---------------- Phase A(g): encoder + candidates ---------
            xhi_sb = xpool.tile([128, NDC, GB], bf16, tag="xhi",
                                name=f"xhi{g}")
            nc.sync.dma_start(
                out=xhi_sb,
                in_=xhi[:, gc].rearrange("(j p) b -> p j b", p=128))
            xlo_sb = xpool.tile([128, NDC, GB], bf16, tag="xlo",
                                name=f"xlo{g}")
            nc.sync.dma_start(
                out=xlo_sb,
                in_=xlo[:, gc].rearrange("(j p) b -> p j b", p=128))
            cands = [candpool.tile([128, NSEG * 8], fp32, tag="cand",
                                   name=f"cand{g}_{t}")
                     for t in range(NBT)]
            for c in range(NHC):
                hs = slice(c * HCH, (c + 1) * HCH)
                whi_sb = wpool.tile([128, NDC, HCH], bf16, tag="whi",
                                    name=f"whi{g}_{c}")
                nc.sync.dma_start(
                    out=whi_sb,
                    in_=whi[:, hs].rearrange("(j p) h -> p j h", p=128))
                wlo_sb = wpool.tile([128, NDC, HCH], bf16, tag="wlo",
                                    name=f"wlo{g}_{c}")
                nc.sync.dma_start(
                    out=wlo_sb,
                    in_=wlo[:, hs].rearrange("(j p) h -> p j h", p=128))
                bst_sb = wpool.tile([2, HCH], bf16, tag="bst",
                                    name=f"bst{g}_{c}")
                nc.sync.dma_start(out=bst_sb, in_=bstack[:, hs])
                for t in range(NBT):
                    ts_ = slice(t * 128, (t + 1) * 128)
                    ps = psA.tile([128, HCH], fp32, tag="psa",
                                  name=f"psa{g}_{c}_{t}")
                    for d in range(NDC):
                        nc.tensor.matmul(ps, lhsT=xhi_sb[:, d, ts_],
                                         rhs=whi_sb[:, d, :],
                                         start=(d == 0), stop=False)
                        nc.tensor.matmul(ps, lhsT=xhi_sb[:, d, ts_],
                                         rhs=wlo_sb[:, d, :],
                                         start=False, stop=False)
                        nc.tensor.matmul(ps, lhsT=xlo_sb[:, d, ts_],
                                         rhs=whi_sb[:, d, :],
                                         start=False, stop=False)
                    nc.tensor.matmul(ps, lhsT=ones2, rhs=bst_sb,
                                     start=False, stop=True)
                    ch = cpool.tile([128, HCH], fp32, tag="encch",
                                    name=f"ch{g}_{c}_{t}")
                    nc.scalar.activation(ch, ps, RELU)
                    nc.scalar.dma_start(out=enc_dram[g][ts_, hs], in_=ch)
                    # per-128-segment top-8 candidates while chunk is hot
                    for si in range(HCH // SEG):
                        sgi = c * (HCH // SEG) + si
                        nc.vector.max(
                            out=cands[t][:, sgi * 8:(sgi + 1) * 8],
                            in_=ch[:, si * SEG:(si + 1) * SEG])

            # ---------------- Phase B(g): threshold + mask + transpose --
            v32s = []
            for t in range(NBT):
                cand = cands[t]
                mlast = None
                for r in range(K // 8):
                    m = mpool.tile([128, 8], fp32, tag="m8",
                                   name=f"m{g}_{t}_{r}")
                    nc.vector.max(out=m, in_=cand)
                    if r < K // 8 - 1:
                        nc.vector.match_replace(out=cand, in_to_replace=m,
                                                in_values=cand,
                                                imm_value=0.0)
                    mlast = m
                v32s.append(mlast[:, 7:8])
            # quarter-major so all tiles' quarter q land before q+1:
            # lets the decoder start on h-chunks of quarter 0 early
            for q in range(4):
                qs = slice(q * HQ, (q + 1) * HQ)
                for t in range(NBT):
                    ts_ = slice(t * 128, (t + 1) * 128)
                    eq = eqpool.tile([128, HQ], fp32, tag="eq",
                                     name=f"eq{g}_{t}_{q}")
                    eng = (nc.sync, nc.gpsimd, nc.scalar, nc.sync)[t]
                    eng.dma_start(out=eq, in_=enc_dram[g][ts_, qs])
                    nc.vector.scalar_tensor_tensor(
                        out=eq, in0=eq, scalar=v32s[t], in1=eq,
                        op0=mybir.AluOpType.is_ge, op1=mybir.AluOpType.mult)
                    eng2 = (nc.gpsimd, nc.scalar, nc.sync, nc.gpsimd)[t]
                    eng2.dma_start(
                        out=enc_out[g * GB + t * 128:g * GB + (t + 1) * 128,
                                    qs],
                        in_=eq)
                    encT_sb = etpool.tile([128, 32, 128], bf16, tag="encT",
                                          name=f"encT{g}_{t}_{q}")
                    for j in range(32):
                        pst = psT.tile([128, 128], fp32, tag="pst",
                                       name=f"pst{g}_{t}_{q}_{j}")
                        nc.tensor.transpose(
                            pst, eq[:, j * 128:(j + 1) * 128], ident)
                        nc.scalar.activation(encT_sb[:, j, :], pst, COPY)
                    nc.sync.dma_start(
                        out=encT_dram[g][q].rearrange(
                            "(j p) b -> p j b", p=128)[:, :, ts_],
                        in_=encT_sb)

            # ---------------- Phase C(g): decoder ----------------
            for oh in range(2):
                os_ = slice(oh * 512, (oh + 1) * 512)
                pss = [psC.tile([128, 512], fp32, tag="psdec",
                                name=f"psdec{g}_{oh}_{i}")
                       for i in range(NBT)]
                for c in range(H // 128):
                    cs = slice(c * 128, (c + 1) * 128)
                    wd = wdpool.tile([128, 512], bf16, tag="wd",
                                     name=f"wd{g}_{oh}_{c}")
                    nc.sync.dma_start(out=wd, in_=wdecT[cs, os_])
                    et = ecpool.tile([128, GB], bf16, tag="et",
                                     name=f"et{g}_{oh}_{c}")
                    nc.sync.dma_start(
                        out=et,
                        in_=encT_dram[g][c // 32][(c % 32) * 128:
                                                  (c % 32 + 1) * 128, :])
                    for t in range(NBT):
                        nc.tensor.matmul(
                            pss[t], lhsT=et[:, t * 128:(t + 1) * 128],
                            rhs=wd, start=(c == 0), stop=False)
                for t in range(NBT):
                    nc.tensor.matmul(pss[t], lhsT=ones_f32,
                                     rhs=bdec_sb[:, os_],
                                     start=False, stop=True)
                    do = dpool.tile([128, 512], fp32, tag="do",
                                    name=f"do{g}_{oh}_{t}")
                    nc.scalar.activation(do, pss[t], COPY)
                    nc.scalar.dma_start(
                        out=dec_out[g * GB + t * 128:g * GB + (t + 1) * 128,
                                    os_],
                        in_=do)

    nc.compile()
    return nc


def _split_bf16(a):
    hi = a.astype(ml_dtypes.bfloat16)
    lo = (a - hi.astype(np.float32)).astype(ml_dtypes.bfloat16)
    return hi, lo


def kernel(x, W_enc, b_enc, W_dec, b_dec, topk):
    assert int(topk) == K
    from concourse.bass_utils import run_bass_kernel_spmd

    x = np.asarray(x, dtype=np.float32)
    W_enc = np.asarray(W_enc, dtype=np.float32)
    b_enc = np.asarray(b_enc, dtype=np.float32)
    W_dec = np.asarray(W_dec, dtype=np.float32)
    b_dec = np.asarray(b_dec, dtype=np.float32)

    if "nc" not in _cache:
        _cache["nc"] = _build()
    nc = _cache["nc"]

    xT = np.ascontiguousarray(x.T)  # [D, B]
    xT_hi, xT_lo = _split_bf16(xT)
    wencT = np.ascontiguousarray(W_enc.T)  # [D, H]
    w_hi, w_lo = _split_bf16(wencT)
    b_hi, b_lo = _split_bf16(b_enc.reshape(1, H))
    bstack = np.ascontiguousarray(np.concatenate([b_hi, b_lo], axis=0))
    wdecT = np.ascontiguousarray(W_dec.T).astype(ml_dtypes.bfloat16)
    bdec = np.ascontiguousarray(b_dec.reshape(1, O))

    in_maps = []
    for c in range(NCORES):
        cs = slice(c * BSH, (c + 1) * BSH)
        in_maps.append({
            "xhi": np.ascontiguousarray(xT_hi[:, cs]),
            "xlo": np.ascontiguousarray(xT_lo[:, cs]),
            "whi": w_hi,
            "wlo": w_lo,
            "bstack": bstack,
            "wdecT": wdecT,
            "bdec": bdec,
        })

    res = run_bass_kernel_spmd(nc, in_maps, core_ids=list(range(NCORES)),
                               trace=TRACE)
    LAST_RESULTS["exec_time_ns"] = res.exec_time_ns
    LAST_RESULTS["profile_json"] = res.profile_json

    enc_sparse = np.concatenate([res.results[c]["enc_sparse"]
                                 for c in range(NCORES)], axis=0)
    dec = np.concatenate([res.results[c]["dec"]
                          for c in range(NCORES)], axis=0)
    return enc_sparse.astype(np.float32), dec.astype(np.float32)


# revision 18
# speedup vs baseline: 1.0641x; 1.0641x over previous
"""Trainium2 Bass kernel for sparse autoencoder (topk masking).

  encoder:  pre = x @ W_enc.T + b_enc ; enc = relu(pre)
  topk:     per-row top-32 of enc kept, rest zeroed  -> encoded_sparse
  decoder:  dec = encoded_sparse @ W_dec.T + b_dec
  returns (encoded_sparse, dec)

Sharding: pure data-parallel over the batch dim across 8 NeuronCores
(1024 rows per core).  No collectives needed.

v6 pipeline per core (2 batch groups of 512 rows for cross-phase
overlap):
  Phase A(g): bf16 hi/lo-split matmuls (fp32-accurate), fused ReLU on
      ScalarE, park dense enc rows in DRAM.  While each relu chunk is
      still in SBUF, VectorE computes per-128-column-segment top-8
      candidates (no extra HBM traffic).
  Phase B(g): per row-tile: 7 small ops on the 1024-wide candidate
      array give the exact 32nd-largest value v32; then a streaming
      quarter-pipeline re-reads enc, applies the fused
      (enc >= v32) * enc mask in one VectorE pass, writes
      encoded_sparse, and PE-transposes to bf16 encT for the decoder.
  Phase C(g): dense bf16 decode, W_decT streamed, PSUM accumulation
      over 128 h-chunks, bias via K=1 matmul.
"""

import sys

sys.path.insert(0, "/opt/trn_rl_repo")

import numpy as np
import ml_dtypes

B, D, H, O, K = 8192, 1024, 16384, 1024, 32
NCORES = 8
BSH = B // NCORES  # 1024 rows per core
NG = 2             # batch groups per core
GB = BSH // NG     # 512 rows per group
NBT = GB // 128    # 4 row-tiles per group
HCH = 512          # encoder h-chunk
NHC = H // HCH     # 32
NDC = D // 128     # 8 contraction chunks
SEG = 128          # top-k candidate segment width
NSEG = H // SEG    # 128 segments -> 1024 candidates/row
HQ = H // 4        # phase-B quarter width (4096)

TRACE = False
LAST_RESULTS = {}

_cache = {}


def _build():
    import concourse.bass as bass  # noqa: F401
    import concourse.mybir as mybir
    import concourse.tile as tile
    from concourse import bacc
    from concourse.masks import make_identity
    from contextlib import ExitStack

    fp32 = mybir.dt.float32
    bf16 = mybir.dt.bfloat16
    RELU = mybir.ActivationFunctionType.Relu
    COPY = mybir.ActivationFunctionType.Copy

    nc = bacc.Bacc("TRN2", target_bir_lowering=False, debug=False,
                   num_devices=NCORES)

    xhi = nc.dram_tensor("xhi", [D, BSH], bf16, kind="ExternalInput").ap()
    xlo = nc.dram_tensor("xlo", [D, BSH], bf16, kind="ExternalInput").ap()
    whi = nc.dram_tensor("whi", [D, H], bf16, kind="ExternalInput").ap()
    wlo = nc.dram_tensor("wlo", [D, H], bf16, kind="ExternalInput").ap()
    bstack = nc.dram_tensor("bstack", [2, H], bf16, kind="ExternalInput").ap()
    wdecT = nc.dram_tensor("wdecT", [H, O], bf16, kind="ExternalInput").ap()
    bdec = nc.dram_tensor("bdec", [1, O], fp32, kind="ExternalInput").ap()
    enc_out = nc.dram_tensor("enc_sparse", [BSH, H], fp32,
                             kind="ExternalOutput").ap()
    dec_out = nc.dram_tensor("dec", [BSH, O], fp32,
                             kind="ExternalOutput").ap()

    with tile.TileContext(nc) as tc, ExitStack() as ctx:
        const = ctx.enter_context(tc.tile_pool(name="const", bufs=1))
        dram = ctx.enter_context(tc.tile_pool(name="dram", bufs=1,
                                              space="DRAM"))
        xpool = ctx.enter_context(tc.tile_pool(name="xTp", bufs=1))
        wpool = ctx.enter_context(tc.tile_pool(name="wenc", bufs=2))
        cpool = ctx.enter_context(tc.tile_pool(name="encch", bufs=3))
        candpool = ctx.enter_context(tc.tile_pool(name="cand", bufs=8))
        eqpool = ctx.enter_context(tc.tile_pool(name="encq", bufs=2))
        etpool = ctx.enter_context(tc.tile_pool(name="encT", bufs=6))
        mpool = ctx.enter_context(tc.tile_pool(name="m8", bufs=8))
        wdpool = ctx.enter_context(tc.tile_pool(name="wdec", bufs=3))
        dapool = ctx.enter_context(tc.tile_pool(name="dacc", bufs=4))
        psA = ctx.enter_context(tc.tile_pool(name="psA", bufs=2,
                                             space="PSUM"))
        psT = ctx.enter_context(tc.tile_pool(name="psT", bufs=2,
                                             space="PSUM"))
        psC = ctx.enter_context(tc.tile_pool(name="psC", bufs=4,
                                             space="PSUM"))

        ident = const.tile([128, 128], fp32)
        make_identity(nc, ident)
        ones2 = const.tile([2, 128], bf16)
        nc.vector.memset(ones2, 1.0)
        ones_f32 = const.tile([1, 128], fp32)
        nc.vector.memset(ones_f32, 1.0)
        bdec_sb = const.tile([1, O], fp32)
        nc.sync.dma_start(out=bdec_sb, in_=bdec)
        bias_bc = const.tile([128, O], fp32)
        for _bh in range(2):
            _bps = psC.tile([128, 512], fp32, tag="psdec",
                            name=f"biasps{_bh}")
            nc.tensor.matmul(_bps, lhsT=ones_f32,
                             rhs=bdec_sb[:, _bh * 512:(_bh + 1) * 512],
                             start=True, stop=True)
            nc.scalar.activation(bias_bc[:, _bh * 512:(_bh + 1) * 512],
                                 _bps, COPY)

        enc_dram = [dram.tile([GB, H], fp32, name=f"enc_dram{g}")
                    for g in range(NG)]


        for g in range(NG):
            gc = slice(g * GB, (g + 1) * GB)  # this group's batch columns
            # ---------------- Phase A(g): encoder + candidates ---------
            xhi_sb = xpool.tile([128, NDC, GB], bf16, tag="xhi",
                                name=f"xhi{g}")
            nc.sync.dma_start(
                out=xhi_sb,
                in_=xhi[:, gc].rearrange("(j p) b -> p j b", p=128))
            xlo_sb = xpool.tile([128, NDC, GB], bf16, tag="xlo",
                                name=f"xlo{g}")
            nc.sync.dma_start(
                out=xlo_sb,
                in_=xlo[:, gc].rearrange("(j p) b -> p j b", p=128))
            cands = [candpool.tile([128, NSEG * 8], fp32, tag="cand",
                                   name=f"cand{g}_{t}")
                     for t in range(NBT)]
            for c in range(NHC):
                hs = slice(c * HCH, (c + 1) * HCH)
                whi_sb = wpool.tile([128, NDC, HCH], bf16, tag="whi",
                                    name=f"whi{g}_{c}")
                nc.sync.dma_start(
                    out=whi_sb,
                    in_=whi[:, hs].rearrange("(j p) h -> p j h", p=128))
                wlo_sb = wpool.tile([128, NDC, HCH], bf16, tag="wlo",
                                    name=f"wlo{g}_{c}")
                nc.sync.dma_start(
                    out=wlo_sb,
                    in_=wlo[:, hs].rearrange("(j p) h -> p j h", p=128))
                bst_sb = wpool.tile([2, HCH], bf16, tag="bst",
                                    name=f"bst{g}_{c}")
                nc.sync.dma_start(out=bst_sb, in_=bstack[:, hs])
                for t in range(NBT):
                    ts_ = slice(t * 128, (t + 1) * 128)
                    ps = psA.tile([128, HCH], fp32, tag="psa",
                                  name=f"psa{g}_{c}_{t}")
                    for d in range(NDC):
                        nc.tensor.matmul(ps, lhsT=xhi_sb[:, d, ts_],
                                         rhs=whi_sb[:, d, :],
                                         start=(d == 0), stop=False)
                        nc.tensor.matmul(ps, lhsT=xhi_sb[:, d, ts_],
                                         rhs=wlo_sb[:, d, :],
                                         start=False, stop=False)
                        nc.tensor.matmul(ps, lhsT=xlo_sb[:, d, ts_],
                                         rhs=whi_sb[:, d, :],
                                         start=False, stop=False)
                    nc.tensor.matmul(ps, lhsT=ones2, rhs=bst_sb,
                                     start=False, stop=True)
                    ch = cpool.tile([128, HCH], fp32, tag="encch",
                                    name=f"ch{g}_{c}_{t}")
                    nc.scalar.activation(ch, ps, RELU)
                    nc.scalar.dma_start(out=enc_dram[g][ts_, hs], in_=ch)
                    # per-128-segment top-8 candidates while chunk is hot
                    for si in range(HCH // SEG):
                        sgi = c * (HCH // SEG) + si
                        nc.vector.max(
                            out=cands[t][:, sgi * 8:(sgi + 1) * 8],
                            in_=ch[:, si * SEG:(si + 1) * SEG])

            # ---- Phase B(g)+C(g): threshold, mask, transpose, decode ----
            for t in range(NBT):
                cand = cands[t]
                mlast = None
                for r in range(K // 8):
                    m = mpool.tile([128, 8], fp32, tag="m8",
                                   name=f"m{g}_{t}_{r}")
                    nc.vector.max(out=m, in_=cand)
                    if r < K // 8 - 1:
                        nc.vector.match_replace(out=cand, in_to_replace=m,
                                                in_values=cand,
                                                imm_value=0.0)
                    mlast = m
                if t == 0:
                    v32s = []
                v32s.append(mlast[:, 7:8])
            daccs = [dapool.tile([128, O], fp32, tag="dacc",
                                 name=f"dacc{g}_{t}")
                     for t in range(NBT)]
            for q in range(4):
                qs = slice(q * HQ, (q + 1) * HQ)
                encTs = []
                for t in range(NBT):
                    ts_ = slice(t * 128, (t + 1) * 128)
                    eq = eqpool.tile([128, HQ], fp32, tag="eq",
                                     name=f"eq{g}_{t}_{q}")
                    eng = (nc.sync, nc.gpsimd, nc.scalar, nc.sync)[t]
                    eng.dma_start(out=eq, in_=enc_dram[g][ts_, qs])
                    nc.vector.scalar_tensor_tensor(
                        out=eq, in0=eq, scalar=v32s[t], in1=eq,
                        op0=mybir.AluOpType.is_ge, op1=mybir.AluOpType.mult)
                    eng2 = (nc.gpsimd, nc.scalar, nc.sync, nc.gpsimd)[t]
                    eng2.dma_start(
                        out=enc_out[g * GB + t * 128:g * GB + (t + 1) * 128,
                                    qs],
                        in_=eq)
                    encT_sb = etpool.tile([128, 32, 128], bf16, tag="encT",
                                          name=f"encT{g}_{t}_{q}")
                    for j in range(32):
                        pst = psT.tile([128, 128], fp32, tag="pst",
                                       name=f"pst{g}_{t}_{q}_{j}")
                        nc.tensor.transpose(
                            pst, eq[:, j * 128:(j + 1) * 128], ident)
                        nc.scalar.activation(encT_sb[:, j, :], pst, COPY)
                    encTs.append(encT_sb)
                # decode this h-quarter from SBUF-resident encT
                for oh in range(2):
                    os_ = slice(oh * 512, (oh + 1) * 512)
                    pss = [psC.tile([128, 512], fp32, tag="psdec",
                                    name=f"psdec{g}_{q}_{oh}_{i}")
                           for i in range(NBT)]
                    for cc in range(32):
                        cs = slice(q * HQ // 128 * 128 + cc * 128,
                                   q * HQ + (cc + 1) * 128)
                        wd = wdpool.tile([128, 512], bf16, tag="wd",
                                         name=f"wd{g}_{q}_{oh}_{cc}")
                        nc.sync.dma_start(
                            out=wd,
                            in_=wdecT[q * HQ + cc * 128:
                                      q * HQ + (cc + 1) * 128, os_])
                        for t in range(NBT):
                            nc.tensor.matmul(
                                pss[t], lhsT=encTs[t][:, cc, :], rhs=wd,
                                start=(cc == 0), stop=(cc == 31))
                    for t in range(NBT):
                        if q == 0:
                            nc.vector.tensor_add(daccs[t][:, os_], pss[t],
                                                 bias_bc[:, os_])
                        else:
                            nc.vector.tensor_add(daccs[t][:, os_],
                                                 daccs[t][:, os_], pss[t])
            for t in range(NBT):
                nc.scalar.dma_start(
                    out=dec_out[g * GB + t * 128:g * GB + (t + 1) * 128, :],
                    in_=daccs[t])

    nc.compile()
    return nc


def _split_bf16(a):
    hi = a.astype(ml_dtypes.bfloat16)
    lo = (a - hi.astype(np.float32)).astype(ml_dtypes.bfloat16)
    return hi, lo


def kernel(x, W_enc, b_enc, W_dec, b_dec, topk):
    assert int(topk) == K
    from concourse.bass_utils import run_bass_kernel_spmd

    x = np.asarray(x, dtype=np.float32)
    W_enc = np.asarray(W_enc, dtype=np.float32)
    b_enc = np.asarray(b_enc, dtype=np.float32)
    W_dec = np.asarray(W_dec, dtype=np.float32)
    b_dec = np.asarray(b_dec, dtype=np.float32)

    if "nc" not in _cache:
        _cache["nc"] = _build()
    nc = _cache["nc"]

    xT = np.ascontiguousarray(x.T)  # [D, B]
    xT_hi, xT_lo = _split_bf16(xT)
    wencT = np.ascontiguousarray(W_enc.T)  # [D, H]
    w_hi, w_lo = _split_bf16(wencT)
    b_hi, b_lo = _split_bf16(b_enc.reshape(1, H))
    bstack = np.ascontiguousarray(np.concatenate([b_hi, b_lo], axis=0))
    wdecT = np.ascontiguousarray(W_dec.T).astype(ml_dtypes.bfloat16)
    bdec = np.ascontiguousarray(b_dec.reshape(1, O))

    in_maps = []
    for c in range(NCORES):
        cs = slice(c * BSH, (c + 1) * BSH)
        in_maps.append({
            "xhi": np.ascontiguousarray(xT_hi[:, cs]),
            "xlo": np.ascontiguousarray(xT_lo[:, cs]),
            "whi": w_hi,
            "wlo": w_lo,
            "bstack": bstack,
            "wdecT": wdecT,
            "bdec": bdec,
        })

    res = run_bass_kernel_spmd(nc, in_maps, core_ids=list(range(NCORES)),
                               trace=TRACE)
    LAST_RESULTS["exec_time_ns"] = res.exec_time_ns
    LAST_RESULTS["profile_json"] = res.profile_json

    enc_sparse = np.concatenate([res.results[c]["enc_sparse"]
                                 for c in range(NCORES)], axis=0)
    dec = np.concatenate([res.results[c]["dec"]
                          for c in range(NCORES)], axis=0)
    return enc_sparse.astype(np.float32), dec.astype(np.float32)
